# revision 23
# baseline (speedup 1.0000x reference)
"""Self-contained Trainium2 (Bass/Tile) kernel for nn_CQAttention.

kernel(**inputs) takes FULL inputs (B=64) and returns the FULL output
[64, 2048, 512] (= concat[C, A, C*A, C*Bm]). Internally shards batch across
8 NeuronCores (data parallel, 8 batches/core) and runs a Bass/Tile program
via concourse.bass_utils.run_bass_kernel_spmd.

Math (per batch; bias is a constant shift so it cancels in both softmaxes,
and per-row/per-col shifts cancel in the softmax over that axis):
  s2[c,q] = sum_d C[c,d]*w4mul[d]*Q[q,d];  s0[c] = C@w4c;  s1[q] = Q@w4q
  G[c,q]  = exp(s0[c] + s2[c,q])          # softmax-c numerator (s1 cancels)
  H[q,c]  = exp(s1[q] + s2[c,q])          # softmax-q numerator (s0 cancels)
  M'      = (G^T C) / colsum_c(G)         # = S2^T C
  A       = (H^T)^T Q / d1,  d1[c] = sum_q H[q,c]
  Bm      = (H^T)^T M' / d1
  out     = [C, A, C*A, C*Bm]
The bias terms are applied via the ACT engine's per-partition bias port
(no rank-2 matmuls), and both softmax denominators come out of the M'/A
matmuls as one extra ones-column on the rhs.

Emission is software-pipelined in 3 stages (preamble(s) | E-phase(s-1) |
back-phase(s-2)) with unit-level interleaving so the PE fills the
ACT-gated gaps of the E-phase with back-phase matmuls.
"""
import sys
import numpy as np

for _p in ("/opt/trn_rl_repo",):
    if _p not in sys.path:
        sys.path.insert(0, _p)

import concourse.bass as bass
import concourse.mybir as mybir
import concourse.tile as tile
from concourse import bacc
from concourse.masks import make_identity
from concourse.bass_utils import run_bass_kernel_spmd
from contextlib import ExitStack

F32 = mybir.dt.float32
BF16 = mybir.dt.bfloat16
AF = mybir.ActivationFunctionType
AX = mybir.AxisListType

N_CORES = 8
B, CL, QL, D = 64, 2048, 512, 128
NB = B // N_CORES  # batches per core


def _build_body(nc, tc, ctx, nb, cl, ql, d, C_d, Q_d, w4c_d, w4q_d, w4m_d, OUT_d):
    NT = cl // 128   # 16 c-tiles
    NQ = ql // 128   # 4 q-tiles
    NE = NT + 2 * NQ  # E-phase units: 16 ec + 8 eq(paired half-rows)

    consts = ctx.enter_context(tc.tile_pool(name="consts", bufs=1))
    ident = consts.tile([128, 128], BF16)
    make_identity(nc, ident)
    w4m_sb = consts.tile([d, 1], F32)
    nc.sync.dma_start(w4m_sb, w4m_d)
    # [128, d] broadcast rows of w4c / w4q for the per-row dot products
    w4c_bc = consts.tile([128, d], F32)
    nc.sync.dma_start(w4c_bc, w4c_d.rearrange("d one -> one d")
                      .broadcast_to((128, d)))
    w4q_bc = consts.tile([128, d], F32)
    nc.sync.dma_start(w4q_bc, w4q_d.rearrange("d one -> one d")
                      .broadcast_to((128, d)))
    ones_bf = consts.tile([128, NT], BF16)
    nc.gpsimd.memset(ones_bf, 1.0)

    # SBUF pools (bufs chosen for the 3-slot pipeline lifetimes)
    ld = ctx.enter_context(tc.tile_pool(name="ld", bufs=4))
    bfp = ctx.enter_context(tc.tile_pool(name="bfp", bufs=4))
    mbp = ctx.enter_context(tc.tile_pool(name="mbp", bufs=2))
    tpp = ctx.enter_context(tc.tile_pool(name="tpp", bufs=2))
    stat = ctx.enter_context(tc.tile_pool(name="stat", bufs=2))
    epool = ctx.enter_context(tc.tile_pool(name="epool", bufs=2))
    bstat = ctx.enter_context(tc.tile_pool(name="bstat", bufs=2))
    outp = ctx.enter_context(tc.tile_pool(name="outp", bufs=3))
    absp = ctx.enter_context(tc.tile_pool(name="absp", bufs=3))
    tmpp = ctx.enter_context(tc.tile_pool(name="tmpp", bufs=2))

    # PSUM: ec 2x1 + eq 2x2 + acc 2x1 = 8 banks
    ec_ps = ctx.enter_context(tc.tile_pool(name="ec_ps", bufs=2, space="PSUM"))
    eq_ps = ctx.enter_context(tc.tile_pool(name="eq_ps", bufs=2, space="PSUM"))
    acc_ps = ctx.enter_context(tc.tile_pool(name="acc_ps", bufs=2, space="PSUM"))

    T = {}  # per-batch tile handoff between pipeline stages

    def pre_a(b):
        """Stage A1 (phase 1): HBM loads + Pool c-cast."""
        t = T[b] = {}
        c_nat = t["c_nat"] = ld.tile([128, NT, d], F32, tag="c_nat", name="c_nat")
        nc.sync.dma_start(c_nat, C_d[b].rearrange("(t p) d -> p t d", p=128))
        q_nat = t["q_nat"] = ld.tile([128, NQ, d], F32, tag="q_nat", name="q_nat")
        nc.sync.dma_start(q_nat, Q_d[b].rearrange("(t p) d -> p t d", p=128))
        yield

    def pre_b(b):
        """Stage A2 (phase 2): casts + DVE-heavy prep — runs while phase 2's
        ACT is busy with eq exps and Pool is idle."""
        t = T[b]
        c_nat, q_nat = t["c_nat"], t["q_nat"]
        cm = t["cm"] = bfp.tile([128, NT, d + 1], BF16, tag="cm", name="cm")
        nc.gpsimd.tensor_copy(cm[:, :, 0:d], c_nat)
        nc.gpsimd.tensor_copy(cm[:, :, d], ones_bf[:, 0:NT])
        yield
        qm = t["qm"] = bfp.tile([128, NQ, d + 1], BF16, tag="qm", name="qm")
        nc.vector.tensor_copy(qm[:, :, 0:d], q_nat)
        nc.vector.tensor_copy(qm[:, :, d], ones_bf[:, 0:NQ])
        yield
        # s0[c] in [c-part, t] layout via row-dot on DVE (no transpose needed)
        s0_pt = t["s0_pt"] = stat.tile([128, NT], F32, tag="s0", name="s0_pt")
        tmp = tmpp.tile([128, NT, d], BF16, tag="tmp")
        nc.vector.tensor_mul(tmp, c_nat,
                             w4c_bc.unsqueeze(1).broadcast_to((128, NT, d)))
        yield
        nc.vector.reduce_sum(s0_pt, tmp, axis=AX.X)
        yield
        s1_pt = t["s1_pt"] = stat.tile([128, NQ], F32, tag="s1", name="s1_pt")
        tmq = tmpp.tile([128, NQ, d], BF16, tag="tmq")
        nc.vector.tensor_mul(tmq, q_nat,
                             w4q_bc.unsqueeze(1).broadcast_to((128, NQ, d)))
        nc.vector.reduce_sum(s1_pt, tmq, axis=AX.X)
        yield
        # PE transposes (bf16, 1 cyc/row), drains on DVE (2x bf16)
        ct = t["ct"] = tpp.tile([128, NT, d], BF16, tag="ct", name="ct")
        for g in range(NT // 4):
            tp = ec_ps.tile([128, 4, 128], BF16, tag="ec", name="tp")
            for i in range(4):
                nc.tensor.transpose(tp[:, i, :], cm[:, g * 4 + i, 0:d], ident)
            nc.vector.tensor_copy(ct[:, g * 4:(g + 1) * 4, :], tp)
            yield
        qt = t["qt"] = tpp.tile([128, NQ, d], BF16, tag="qt", name="qt")
        tpq = ec_ps.tile([128, 4, 128], BF16, tag="ec", name="tpq")
        for i in range(NQ):
            nc.tensor.transpose(tpq[:, i, :], qm[:, i, 0:d], ident)
        nc.vector.tensor_copy(qt, tpq)
        yield
        qwt = t["qwt"] = tpp.tile([128, NQ, d], BF16, tag="qwt", name="qwt")
        nc.vector.tensor_scalar_mul(qwt.rearrange("p a b -> p (a b)"),
                                    qt.rearrange("p a b -> p (a b)"), w4m_sb)
        yield

    def ephase_ec(b):
        """Stage B1: ec = exp(s0 + s2) in [c,q] layout, 16 units."""
        t = T[b]
        ct, qwt = t["ct"], t["qwt"]
        s0_pt = t["s0_pt"]
        qwt_flat = qwt.rearrange("p a b -> p (a b)")
        ec = t["ec"] = epool.tile([128, NT, ql], BF16, tag="ec", name="ec")
        for ti in range(NT):
            epc = ec_ps.tile([128, 512], F32, tag="ec", name="epc")
            nc.tensor.matmul(epc, ct[:, ti, :], qwt_flat)
            nc.scalar.activation(ec[:, ti, :], epc, AF.Exp,
                                 bias=s0_pt[:, ti:ti + 1])
            yield

    def ephase_eq(b):
        """Stage B2: eq = exp(s1 + s2^T) in [q,c] layout, 8 paired units."""
        t = T[b]
        ct, qwt = t["ct"], t["qwt"]
        s1_pt = t["s1_pt"]
        ct_flat = ct.rearrange("p a b -> p (a b)")
        eq = t["eq"] = epool.tile([128, NQ, cl], BF16, tag="eq", name="eq")
        for hi in range(2 * NQ):
            qi, half = hi // 2, hi % 2
            epq = eq_ps.tile([128, 1024], F32, tag="eq", name="epq")
            base = half * 1024
            nc.tensor.matmul(epq[:, 0:512], qwt[:, qi, :],
                             ct_flat[:, base:base + 512])
            nc.tensor.matmul(epq[:, 512:1024], qwt[:, qi, :],
                             ct_flat[:, base + 512:base + 1024])
            nc.scalar.activation(eq[:, qi, base:base + 1024],
                                 epq, AF.Exp, bias=s1_pt[:, qi:qi + 1])
            yield

    def back_mp(b):
        """Stage C1: M' (+colsum) -> m_bf; runs in the tail of b's E-slot."""
        t = T[b]
        ec, cm = t["ec"], t["cm"]
        rcol = bstat.tile([128, NQ], F32, tag="rcol")
        m_bf = t["m_bf"] = mbp.tile([128, NQ, d], BF16, tag="m_bf", name="m_bf")
        for qi in range(NQ):
            mp = acc_ps.tile([128, 512], F32, tag="acc", name="mp")
            for tt in range(NT):
                nc.tensor.matmul(mp[:, 0:d + 1],
                                 ec[:, tt, qi * 128:(qi + 1) * 128],
                                 cm[:, tt, :],
                                 start=(tt == 0), stop=(tt == NT - 1))
            nc.vector.reciprocal(rcol[:, qi:qi + 1], mp[:, d:d + 1])
            nc.vector.tensor_scalar_mul(m_bf[:, qi, :], mp[:, 0:d],
                                        rcol[:, qi:qi + 1])
            yield

    def back_ab(b):
        """Stage C2: A/Bm (+d1), output muls and stores."""
        t = T[b]
        eq, qm = t["eq"], t["qm"]
        c_nat = t["c_nat"]
        m_bf = t["m_bf"]
        rrow = bstat.tile([128, NT], F32, tag="rrow")
        out_r = OUT_d[b].rearrange("(t p) n -> p t n", p=128)
        # A/Bm phase: per c-tile, two accumulation groups in one psum bank:
        # cols 0:129 = [H^T Q | d1] (independent of m_bf), cols 129:257 = H^T M'
        ob = None
        abs4 = None
        for tt in range(NT):
            if tt % 4 == 0:
                ob = outp.tile([128, 4, 2 * d], F32, tag="ob")
                abs4 = absp.tile([128, 4, 2, d], F32, tag="absb", name="abs4")
            i = tt % 4
            ab = acc_ps.tile([128, 512], F32, tag="acc", name="ab")
            for qi in range(NQ):
                nc.tensor.matmul(ab[:, 0:d + 1],
                                 eq[:, qi, tt * 128:(tt + 1) * 128],
                                 qm[:, qi, :],
                                 start=(qi == 0), stop=(qi == NQ - 1))
            for qi in range(NQ):
                nc.tensor.matmul(ab[:, d + 1:2 * d + 1],
                                 eq[:, qi, tt * 128:(tt + 1) * 128],
                                 m_bf[:, qi, :],
                                 start=(qi == 0), stop=(qi == NQ - 1))
            nc.vector.reciprocal(rrow[:, tt:tt + 1], ab[:, d:d + 1])
            ab_v = ab[:, 0:2 * (d + 1)].rearrange("p (two x) -> p two x", two=2)
            nc.vector.tensor_mul(
                abs4[:, i, :, :], ab_v[:, :, 0:d],
                rrow[:, tt:tt + 1].unsqueeze(2).broadcast_to((128, 2, d)))
            if i % 2 == 1:
                cn2 = c_nat[:, tt - 1:tt + 1, :]
                nc.gpsimd.tensor_mul(ob[:, i - 1:i + 1, 0:d],
                                     abs4[:, i - 1:i + 1, 0, :], cn2)
                nc.gpsimd.tensor_mul(ob[:, i - 1:i + 1, d:2 * d],
                                     abs4[:, i - 1:i + 1, 1, :], cn2)
            if i == 3:
                g = tt // 4
                nc.sync.dma_start(out_r[:, g * 4:(g + 1) * 4, d:2 * d],
                                  abs4[:, :, 0, :])
                nc.sync.dma_start(out_r[:, g * 4:(g + 1) * 4, 2 * d:4 * d], ob)
                nc.sync.dma_start(out_r[:, g * 4:(g + 1) * 4, 0:d],
                                  c_nat[:, g * 4:(g + 1) * 4, :])
            yield
        del T[b]

    # ---- drive the pipeline ----
    # slot s, phase 1: ec(s-1) | ab(s-2) | pre_a(s)          (16 rounds)
    # slot s, phase 2: eq(s-1) | mp(s-1) | pre_b(s)          (10 rounds)
    def run_rr(gens):
        live = list(gens)
        while live:
            for g in list(live):
                try:
                    next(g)
                except StopIteration:
                    live.remove(g)

    for s in range(nb + 2):
        g1 = []
        if 1 <= s <= nb:
            g1.append(ephase_ec(s - 1))
        if s >= 2:
            g1.append(back_ab(s - 2))
        if s < nb:
            g1.append(pre_a(s))
        run_rr(g1)
        g2 = []
        if 1 <= s <= nb:
            g2.append(ephase_eq(s - 1))
            g2.append(back_mp(s - 1))
        if s < nb:
            g2.append(pre_b(s))
        run_rr(g2)


def build_program(nb=NB):
    nc = bacc.Bacc("TRN2", target_bir_lowering=False, debug=False,
                   num_devices=N_CORES)
    C_d = nc.dram_tensor("C", [nb, CL, D], F32, kind="ExternalInput").ap()
    Q_d = nc.dram_tensor("Q", [nb, QL, D], F32, kind="ExternalInput").ap()
    w4c_d = nc.dram_tensor("w4c", [D, 1], F32, kind="ExternalInput").ap()
    w4q_d = nc.dram_tensor("w4q", [D, 1], F32, kind="ExternalInput").ap()
    w4m_d = nc.dram_tensor("w4mul", [D, 1], F32, kind="ExternalInput").ap()
    OUT_d = nc.dram_tensor("OUT", [nb, CL, 4 * D], F32, kind="ExternalOutput").ap()
    with ExitStack() as ctx:
        tc = ctx.enter_context(tile.TileContext(nc))
        _build_body(nc, tc, ctx, nb, CL, QL, D,
                    C_d, Q_d, w4c_d, w4q_d, w4m_d, OUT_d)
    nc.compile()
    return nc


_PROGRAM_CACHE = {}


def _get_program(nb=NB):
    if nb not in _PROGRAM_CACHE:
        _PROGRAM_CACHE[nb] = build_program(nb)
    return _PROGRAM_CACHE[nb]


def _numpy_fallback(C, Q, c_mask, q_mask, w4c, w4q, w4mul, bias):
    """Exact reference math in numpy (used only if masks are not all-ones)."""
    NEG_INF = -1e30
    out = np.empty((C.shape[0], C.shape[1], 4 * C.shape[2]), np.float32)
    for b in range(C.shape[0]):
        Cb = C[b].astype(np.float64)
        Qb = Q[b].astype(np.float64)
        S = (Cb @ w4c.reshape(-1, 1) + (Qb @ w4q.reshape(-1, 1)).T
             + (Cb * w4mul.reshape(1, -1)) @ Qb.T + float(np.asarray(bias).reshape(-1)[0]))
        qm = q_mask[b].reshape(1, -1)
        cm = c_mask[b].reshape(-1, 1)
        S1l = S * qm + NEG_INF * (1.0 - qm)
        S2l = S * cm + NEG_INF * (1.0 - cm)
        S1 = np.exp(S1l - S1l.max(1, keepdims=True))
        S1 /= S1.sum(1, keepdims=True)
        S2 = np.exp(S2l - S2l.max(0, keepdims=True))
        S2 /= S2.sum(0, keepdims=True)
        A = S1 @ Qb
        Bm = S1 @ (S2.T @ Cb)
        out[b] = np.concatenate([Cb, A, Cb * A, Cb * Bm], axis=1).astype(np.float32)
    return out


def kernel(C, Q, c_mask, q_mask, w4c, w4q, w4mul, bias):
    C = np.ascontiguousarray(np.asarray(C), dtype=np.float32)
    Q = np.ascontiguousarray(np.asarray(Q), dtype=np.float32)
    c_mask = np.asarray(c_mask)
    q_mask = np.asarray(q_mask)
    w4c = np.asarray(w4c, dtype=np.float32)
    w4q = np.asarray(w4q, dtype=np.float32)
    w4mul = np.asarray(w4mul, dtype=np.float32)

    if not (np.all(c_mask == 1.0) and np.all(q_mask == 1.0)):
        return _numpy_fallback(C, Q, c_mask, q_mask, w4c, w4q, w4mul, bias)

    nc = _get_program(NB)
    w4c_r = np.ascontiguousarray(w4c.reshape(D, 1))
    w4q_r = np.ascontiguousarray(w4q.reshape(D, 1))
    w4m_r = np.ascontiguousarray(w4mul.reshape(D, 1))
    in_maps = []
    for c in range(N_CORES):
        sl = slice(c * NB, (c + 1) * NB)
        in_maps.append({
            "C": np.ascontiguousarray(C[sl]),
            "Q": np.ascontiguousarray(Q[sl]),
            "w4c": w4c_r,
            "w4q": w4q_r,
            "w4mul": w4m_r,
        })
    res = run_bass_kernel_spmd(nc, in_maps, core_ids=list(range(N_CORES)))
    out = np.concatenate([res.results[c]["OUT"] for c in range(N_CORES)], axis=0)
    return out
